# revision 11
# baseline (speedup 1.0000x reference)
# Per-sample 256-bin histogram entropy on trn2 (8 cores, data-parallel over batch).
#
# Algorithm (per core, 8 samples of 786432 f32 each):
#   1. DMA f32 tiles in, convert to fp16 arena (ACT engine).
#   2. Per-sample min/max: DVE free-dim reduce + GPSIMD partition_all_reduce.
#   3. t = (x + (-min)) * (256/range)  in [0, 256);  v = min(t, 255.5) mod 16.
#   4. Build step matrices: hi[i] = [t >= 16i], lo[j] = [v >= j], i,j = 0..15 (bf16 0/1),
#      laid out in "element-slot" form: col m = 8*i + e for 8 element slots.
#   5. PE matmuls accumulate O[m,n] = sum_k hi_steps[k,m] * lo_steps[k,n] into PSUM:
#      diagonal slots (8i+e, 8j+e) give C[i,j] = #{hi >= i AND lo >= j} (2D cumulative counts).
#   6. Host: 2D difference of C -> 256-bin histogram -> entropy -> mean over 64 samples.
#
# The 2D-cumulative/step trick avoids any floor() on device: [t >= 16i] <=> [floor(t/16) >= i].
import numpy as np

P = 128          # SBUF partitions
NB = 16          # bins per level (16 hi x 16 lo = 256)
ES = 8           # element slots per matmul column block
NCORES = 8
BATCH = 64
SPC = BATCH // NCORES          # samples per core
NPS = 3 * 512 * 512            # elements per sample
FPS = NPS // P                 # free-dim length per sample = 6144


def build_nc(spc=SPC, fps=FPS, w=512, ch=2048, debug_taps=False, cvt_bias=-0.5 + 2**-16):
    # cvt_bias: pre-shift before the f32->int16 convert in the floor(t/16) pass.
    # HW converts round-to-nearest -> -0.5+eps gives floor; the CoreSim truncates
    # -> pass +eps instead when simulating.
    """Build the Bass program. w = macro-tile width (cols), ch = DMA chunk cols."""
    import concourse.bacc as bacc
    import concourse.mybir as mybir
    import concourse.tile as tile
    from concourse import bass_isa

    assert fps % w == 0 and w % ES == 0 and fps % ch == 0
    g = w // ES                # matmul groups per macro-tile
    nmacro = fps // w
    f32 = mybir.dt.float32
    f16 = mybir.dt.float16
    bf16 = mybir.dt.bfloat16
    i16 = mybir.dt.int16
    Alu = mybir.AluOpType
    Act = mybir.ActivationFunctionType
    X = mybir.AxisListType.X

    nc = bacc.Bacc(None, target_bir_lowering=False, debug=False)
    x_in = nc.declare_dram_parameter("x", [spc, P, fps], f32, isOutput=False)
    c_out = nc.declare_dram_parameter("cmat", [spc, P, P], f32, isOutput=True)
    if debug_taps:
        g0 = w // ES
        tt_out = nc.declare_dram_parameter("tt_dbg", [P, w], f16, isOutput=True)
        vv_out = nc.declare_dram_parameter("vv_dbg", [P, w], f16, isOutput=True)
        hi_out = nc.declare_dram_parameter("hi_dbg", [P, g0, P], bf16, isOutput=True)
        lo_out = nc.declare_dram_parameter("lo_dbg", [P, g0, P], bf16, isOutput=True)
        sc_out = nc.declare_dram_parameter("sc_dbg", [P, 5], f32, isOutput=True)

    with tile.TileContext(nc) as tc:
        with (
            tc.tile_pool(name="stage", bufs=3) as stage_pool,
            tc.tile_pool(name="xf", bufs=2) as x_pool,
            tc.tile_pool(name="tv", bufs=3) as tv_pool,
            tc.tile_pool(name="slab", bufs=2) as slab_pool,
            tc.tile_pool(name="small", bufs=2) as small_pool,
            tc.tile_pool(name="co", bufs=2) as co_pool,
            tc.tile_pool(name="psum", bufs=2, space="PSUM") as psum_pool,
        ):
            for s in range(spc):
                # ---- phase A: load + f32->fp16 convert + per-sample min/max ----
                xt = x_pool.tile([P, fps], f16, tag="xt")
                for c in range(0, fps, ch):
                    st = stage_pool.tile([P, ch], f32, tag="st")
                    nc.sync.dma_start(out=st[:], in_=x_in[s, :, c : c + ch])
                    nc.scalar.activation(xt[:, c : c + ch], st[:], Act.Copy)
                mx = small_pool.tile([P, 1], f32, tag="mx")
                mn = small_pool.tile([P, 1], f32, tag="mn")
                nc.vector.tensor_reduce(mx[:], xt[:], axis=X, op=Alu.max)
                nc.vector.tensor_reduce(mn[:], xt[:], axis=X, op=Alu.min)
                nmn = small_pool.tile([P, 1], f32, tag="nmn")
                nc.vector.tensor_scalar_mul(nmn[:], mn[:], -1.0)
                # cross-partition: all partitions end up with the global value
                mxr = small_pool.tile([P, 1], f32, tag="mxr")
                nmnr = small_pool.tile([P, 1], f32, tag="nmnr")
                nc.gpsimd.partition_all_reduce(
                    mxr[:], mx[:], channels=P, reduce_op=bass_isa.ReduceOp.max
                )
                nc.gpsimd.partition_all_reduce(
                    nmnr[:], nmn[:], channels=P, reduce_op=bass_isa.ReduceOp.max
                )
                rng = small_pool.tile([P, 1], f32, tag="rng")
                nc.vector.tensor_tensor(rng[:], mxr[:], nmnr[:], op=Alu.add)
                rcp = small_pool.tile([P, 1], f32, tag="rcp")
                nc.vector.reciprocal(rcp[:], rng[:])
                sc = small_pool.tile([P, 1], f32, tag="sc")
                nc.vector.tensor_scalar_mul(sc[:], rcp[:], 256.0)

                # ---- phase B: binning ----
                cm = psum_pool.tile([P, P], f32, tag="cm")
                for m in range(nmacro):
                    xs = xt[:, m * w : (m + 1) * w]
                    tt = tv_pool.tile([P, w], f16, tag="tt")
                    hi16 = tv_pool.tile([P, w], i16, tag="hi16")
                    vv = tv_pool.tile([P, w], f16, tag="vv")
                    # t = (x + nmn) * sc in [0, 256]
                    nc.vector.tensor_scalar(
                        tt[:], xs, nmnr[:], sc[:], op0=Alu.add, op1=Alu.mult
                    )
                    # floor(t/16) via round-nearest int convert; clamp to 15 so the
                    # x == max group (t = 256) gets v = 16 -> fine bin 15.
                    nc.vector.tensor_scalar(
                        hi16[:], tt[:], 0.0625, cvt_bias, op0=Alu.mult, op1=Alu.add
                    )
                    nc.vector.tensor_scalar_min(hi16[:], hi16[:], 15)
                    # v = t - 16*floor(t/16) in [0, 16]
                    nc.vector.scalar_tensor_tensor(
                        out=vv[:], in0=hi16[:], scalar=-16.0, in1=tt[:],
                        op0=Alu.mult, op1=Alu.add,
                    )
                    hi_slab = slab_pool.tile([P, g, P], bf16, tag="hi")
                    lo_slab = slab_pool.tile([P, g, P], bf16, tag="lo")
                    t3 = tt[:].rearrange("p (g e) -> p g e", e=ES)
                    v3 = vv[:].rearrange("p (g e) -> p g e", e=ES)
                    for i in range(NB):
                        thr_hi = 16.0 * i if i else -1.0
                        thr_lo = float(i) if i else -1.0
                        nc.vector.tensor_scalar(
                            hi_slab[:, :, ES * i : ES * (i + 1)],
                            t3, thr_hi, None, op0=Alu.is_ge,
                        )
                        nc.vector.tensor_scalar(
                            lo_slab[:, :, ES * i : ES * (i + 1)],
                            v3, thr_lo, None, op0=Alu.is_ge,
                        )
                    if debug_taps and s == 0 and m == 0:
                        nc.sync.dma_start(out=tt_out[:], in_=tt[:])
                        nc.sync.dma_start(out=vv_out[:], in_=vv[:])
                        nc.sync.dma_start(out=hi_out[:], in_=hi_slab[:])
                        nc.sync.dma_start(out=lo_out[:], in_=lo_slab[:])
                        nc.sync.dma_start(out=sc_out[:, 0:1], in_=mn[:])
                        nc.sync.dma_start(out=sc_out[:, 1:2], in_=nmn[:])
                        nc.sync.dma_start(out=sc_out[:, 2:3], in_=nmnr[:])
                        nc.sync.dma_start(out=sc_out[:, 3:4], in_=mxr[:])
                        nc.sync.dma_start(out=sc_out[:, 4:5], in_=sc[:])
                    for gi in range(g):
                        nc.tensor.matmul(
                            cm[:],
                            hi_slab[:, gi, :],
                            lo_slab[:, gi, :],
                            start=(m == 0 and gi == 0),
                            stop=(m == nmacro - 1 and gi == g - 1),
                        )
                co = co_pool.tile([P, P], f32, tag="co")
                nc.scalar.activation(co[:], cm[:], Act.Copy)
                nc.sync.dma_start(out=c_out[s], in_=co[:])
    nc.compile()
    return nc


def postprocess(cmats, n_per_sample):
    """cmats: [nsamples, P, P] f32 matmul outputs -> list of entropies (bits)."""
    ents = []
    for O in cmats:
        O4 = O.reshape(NB, ES, NB, ES)
        C2 = np.einsum("iaja->ij", O4)  # sum diagonal element slots
        Cp = np.zeros((NB + 1, NB + 1))
        Cp[:NB, :NB] = C2
        h = Cp[:NB, :NB] - Cp[1:, :NB] - Cp[:NB, 1:] + Cp[1:, 1:]
        hist = h.reshape(NB * NB)
        total = hist.sum()
        p = hist / total
        nz = p > 0
        ents.append(-(p[nz] * np.log2(p[nz])).sum())
    return ents


_NC_CACHE = {}


def kernel(y_pred: np.ndarray) -> np.ndarray:
    from concourse.bass_utils import run_bass_kernel_spmd

    assert y_pred.shape == (BATCH, 3, 512, 512) and y_pred.dtype == np.float32
    x = np.ascontiguousarray(y_pred).reshape(NCORES, SPC, P, FPS)
    in_maps = [{"x": x[c]} for c in range(NCORES)]
    if "nc" not in _NC_CACHE:
        _NC_CACHE["nc"] = build_nc()
    res = run_bass_kernel_spmd(_NC_CACHE["nc"], in_maps, list(range(NCORES))).results
    ents = []
    for c in range(NCORES):
        ents.extend(postprocess(res[c]["cmat"], NPS))
    return np.array(np.mean(ents), dtype=np.float32)


if __name__ == "__main__":
    import reference

    inputs = reference.setup_inputs()
    y = np.asarray(inputs["y_pred"])
    out = kernel(y)
    print("kernel out:", out)
